# revision 26
# baseline (speedup 1.0000x reference)
"""Trainium2 Bass kernel for nn_CrossAttention (B=4, N=M=1024, C=768, H=12, D=64).

Sharding: pure data-parallel over 8 cores. Core c handles batch b = c // 2 and
query rows [512*(c%2), 512*(c%2)+512). Each core computes K/V for its batch
(duplicated across the 2 cores sharing a batch) so no collectives are needed.

All-bf16 datapath (fp32 PSUM accumulation); bf16 streams the PE at 1 cycle/row
and avoids the fp32r power throttle. Host-side layout:
  xT  [768, 512]   = x[b, n0:n0+512, :].T   (c-major for Q projection)
  yT  [768, 1024]  = y[b].T                 (c-major for K/V projection)
  w*T [768, 768]   = W.T                    (c-major weights)
  ywr [1, 1024]    = yw[b] row (bf16), bp fp32 row (DMA-replicated to 128
                     partitions for the DVE bias add)

Device dataflow (all matmuls bf16 x bf16 -> fp32 PSUM):
  QT[co,n] = sum_c wqT[c,co] xT[c,n]
  KT[co,m] = sum_c wkT[c,co] yT[c,m] + ones-row x ywr (rank-1 bias matmul)
  V[m,cv]  = sum_c yT[c,m] wvT[c,cv]  in a [128, 12, 128] per-chunk layout
             whose cols 64:128 are memset to 1 so the PV matmul's PSUM rows
             64:128 accumulate Z replicated 64x (softmax denominator, free)
  per head PAIR: two K=64 S-matmuls at PE array tile positions (0,0)/(64,0)
  (they execute concurrently on disjoint sub-arrays) into one [128,1024]
  PSUM tile, ONE exp over [128,1024] -> bf16, two PV matmuls.
  1/Z = stock DVE reciprocal on PSUM rows 64:128 (reciprocal_approx_fast is
  broken on this hardware), DVE multiply -> OT bf16.
  out[n,co] = sum_ci OT[ci,n] wpT[ci,co]; + bp via DVE add (replicated row).

Schedule notes (what the trace iterations taught us):
  - DMA: only use APs whose inner contiguous run is large. Each [128, 768]
    weight chunk of a (k p) n view is one linear 393KB region; the p-major
    single-DMA variant (768B segments) runs at ~2 GB/s/engine and starves
    everything.
  - ~8.5 us of throwaway warm-up matmuls at t=0 keep the HAM activity
    monitor at K=8/8 (2.4 GHz) while the weights load.
  - Block hp runs head-pair hp's 8-chunk attention with head-pair hp+1's
    Q/K projections interleaved at chunks 1/3/5 (V projection inside block
    0), so the PE never idles long enough to re-throttle.
  - Projection PSUM->SBUF casts run on the SCALAR engine: they land in the
    exp FIFO right where the PSUM pool rotation needs them, and keep the
    DVE free for the reciprocals (GpSimd cannot read PSUM; a DVE cast
    behind a 3.4 us reciprocal stalled the PE a full 7 us per block).
  - Output projection runs in two 2-bank PSUM waves, ci=5 last, so only
    the last pair's normalize sits on the critical path.
"""

import sys

for _p in ("/opt/trn_rl_repo",):
    if _p not in sys.path:
        sys.path.insert(0, _p)

import numpy as np
from contextlib import ExitStack

import concourse.bass as bass
import concourse.mybir as mybir
import concourse.tile as tile
from concourse import bacc

F32 = mybir.dt.float32
BF16 = mybir.dt.bfloat16

B = 4
N = 1024
M = 1024
C = 768
H = 12
D = 64
NSH = 512            # query rows per core
CK = C // 128        # 6 chunks of the feature dim
MK = M // 128        # 8 chunks of the key dim
HP = H // 2          # 6 head pairs (one KT/QT co-chunk each)
SCALE = D ** -0.5
N_CORES = 8
N_WARM = 36          # warm-up matmuls to keep HAM at 8/8 during loads


def build_bass():
    nc = bacc.Bacc("TRN2", target_bir_lowering=False, debug=False)

    xT = nc.dram_tensor("xT", [C, NSH], BF16, kind="ExternalInput").ap()
    yT = nc.dram_tensor("yT", [C, M], BF16, kind="ExternalInput").ap()
    ywr = nc.dram_tensor("ywr", [1, M], BF16, kind="ExternalInput").ap()
    wqT = nc.dram_tensor("wqT", [C, C], BF16, kind="ExternalInput").ap()
    wkT = nc.dram_tensor("wkT", [C, C], BF16, kind="ExternalInput").ap()
    wvT = nc.dram_tensor("wvT", [C, C], BF16, kind="ExternalInput").ap()
    wpT = nc.dram_tensor("wpT", [C, C], BF16, kind="ExternalInput").ap()
    bpr = nc.dram_tensor("bpr", [1, C], BF16, kind="ExternalInput").ap()
    out = nc.dram_tensor("out", [NSH, C], F32, kind="ExternalOutput").ap()

    # k-major chunk views: each [128, x] chunk is one contiguous DRAM region
    wq_c = wqT.rearrange("(k p) n -> k p n", p=128)
    wk_c = wkT.rearrange("(k p) n -> k p n", p=128)
    wv_c = wvT.rearrange("(k p) n -> k p n", p=128)
    wp_c = wpT.rearrange("(k p) n -> k p n", p=128)
    xT_c = xT.rearrange("(k p) n -> k p n", p=128)
    yT_c = yT.rearrange("(k p) n -> k p n", p=128)
    out_c = out.rearrange("(k p) n -> k p n", p=128)

    with tile.TileContext(nc) as tc, ExitStack() as ctx:
        wpool = ctx.enter_context(tc.tile_pool(name="w", bufs=4))
        cpool = ctx.enter_context(tc.tile_pool(name="const", bufs=1))
        qpool = ctx.enter_context(tc.tile_pool(name="qt", bufs=3))
        kpool = ctx.enter_context(tc.tile_pool(name="kt", bufs=3))
        vpool = ctx.enter_context(tc.tile_pool(name="vs", bufs=MK))
        opool = ctx.enter_context(tc.tile_pool(name="ot", bufs=CK))
        epool = ctx.enter_context(tc.tile_pool(name="es", bufs=3))
        outpool = ctx.enter_context(tc.tile_pool(name="outs", bufs=2))
        zpool = ctx.enter_context(tc.tile_pool(name="z", bufs=4))
        ppool = ctx.enter_context(tc.tile_pool(name="pp", bufs=2, space="PSUM"))
        oppool = ctx.enter_context(tc.tile_pool(name="op", bufs=4, space="PSUM"))

        # ---- PE warm-up: throwaway matmuls with no DMA dependency ----
        wrm = cpool.tile([128, 512], BF16, tag="wrm")
        nc.gpsimd.memset(wrm, 0.0)
        ones = cpool.tile([1, C], BF16, tag="ones")
        nc.gpsimd.memset(ones, 1.0)
        wps = ppool.tile([128, 1024], F32, tag="pp", name="warmps")
        for i in range(N_WARM):
            nc.tensor.matmul(
                wps[:, 0:512], wrm[:, 0:128], wrm,
                start=(i == 0), stop=(i == N_WARM - 1),
            )

        # ---- input loads: contiguous per-chunk DMAs, 2 HWDGE rings ----
        # scalar ring: wq x6 (Q proj gate), yT x6, wp x6
        # sync ring:   xt x6, yw row, wk x6, wv x6, bp replicate
        wq = wpool.tile([128, CK, C], BF16, tag="w", name="wq")
        xt = cpool.tile([128, CK, NSH], BF16, tag="xt")
        for i in range(CK):
            nc.scalar.dma_start(out=wq[:, i, :], in_=wq_c[i])
            nc.sync.dma_start(out=xt[:, i, :], in_=xT_c[i])
        yw_s = cpool.tile([1, M], BF16, tag="yws")
        nc.sync.dma_start(out=yw_s, in_=ywr)
        yt = cpool.tile([128, CK, M], BF16, tag="yt")
        wk = wpool.tile([128, CK, C], BF16, tag="w", name="wk")
        for i in range(CK):
            nc.scalar.dma_start(out=yt[:, i, :], in_=yT_c[i])
            nc.sync.dma_start(out=wk[:, i, :], in_=wk_c[i])
        wv = wpool.tile([128, CK, C], BF16, tag="w", name="wv")
        for i in range(CK):
            nc.sync.dma_start(out=wv[:, i, :], in_=wv_c[i])
        # wp is not needed until the output projection: load it via the
        # gpsimd SWDGE ring so it costs neither HWDGE ring any bandwidth
        # and adds no issue time on the ACT queue.
        wp = wpool.tile([128, CK, C], BF16, tag="w", name="wp")
        for i in range(CK):
            nc.gpsimd.dma_start(out=wp[:, i, :], in_=wp_c[i])
        bp_s = cpool.tile([1, C], BF16, tag="bps")
        nc.sync.dma_start(out=bp_s, in_=bpr)
        # preload the ACT exp table from the memset ones tile: zero DMA
        # dependency, so the table load cannot block the projection casts
        # queued behind it on the ACT engine.
        warm = cpool.tile([1, 8], BF16, tag="warm")
        nc.scalar.activation(
            warm, ones[0:1, 0:8], mybir.ActivationFunctionType.Exp,
            scale=SCALE,
        )

        def qproj_mm(co):
            ps = ppool.tile([128, 1024], F32, tag="pp")
            for ci in range(CK):
                nc.tensor.matmul(
                    ps[:, 0:512],
                    wq[:, ci, co * 128:(co + 1) * 128],
                    xt[:, ci, :],
                    start=(ci == 0),
                    stop=(ci == CK - 1),
                )
            t = qpool.tile([128, NSH], BF16, tag="qt")
            # ACT cast: lands in the exp FIFO early enough to release the
            # proj PSUM banks for the sp rotation (a DVE cast would sit
            # behind the 8.5us of reciprocals and stall the PE).
            nc.scalar.copy(t, ps[:, 0:512])
            return t

        def kproj_mm(co, t, mh):
            ps = ppool.tile([128, 1024], F32, tag="pp")
            sl = slice(mh * 512, (mh + 1) * 512)
            for ci in range(CK):
                nc.tensor.matmul(
                    ps[:, 0:512],
                    wk[:, ci, co * 128:(co + 1) * 128],
                    yt[:, ci, sl],
                    start=(ci == 0),
                    stop=False,
                )
            # += ones-row^T x yw-row: the additive key bias, rank-1
            nc.tensor.matmul(
                ps[:, 0:512],
                ones[:, co * 128:(co + 1) * 128],
                yw_s[:, sl],
                start=False,
                stop=True,
            )
            nc.scalar.copy(t[:, sl], ps[:, 0:512])

        def vproj(mc):
            t = vpool.tile([128, H, 128], BF16, tag="vs")
            nc.gpsimd.memset(t[:, :, 64:128], 1.0)
            ps = ppool.tile([128, 1024], F32, tag="pp")
            for nh in range(2):
                sl = slice(nh * 512, nh * 512 + 384)
                for ci in range(CK):
                    nc.tensor.matmul(
                        ps[:, sl],
                        yt[:, ci, mc * 128:(mc + 1) * 128],
                        wv[:, ci, nh * 384:(nh + 1) * 384],
                        start=(ci == 0),
                        stop=(ci == CK - 1),
                    )
            for nh in range(2):
                src = ps[:, nh * 512:nh * 512 + 384].rearrange(
                    "p (h e) -> p h e", e=64
                )
                nc.vector.tensor_copy(t[:, nh * 6:(nh + 1) * 6, 0:64], src)
            return t

        vt = [None] * MK
        ot = [None] * HP
        qt = [None] * HP
        kt = [None] * HP

        def attn_block(hp, build_v, build_next):
            """Head-pair hp's attention; next pair's projections (and, for
            hp==0, the V projection) sliced into the chunk loop.  The PV
            pair for chunk mc is emitted during chunk mc+1 so the PE never
            sits behind the in-flight exp."""
            h0, h1 = 2 * hp, 2 * hp + 1
            qtile, ktile = qt[hp], kt[hp]
            op0 = oppool.tile([128, 512], F32, tag="op", name=f"op{h0}")
            op1 = oppool.tile([128, 512], F32, tag="op", name=f"op{h1}")
            nxt = hp + 1
            ess = [None] * MK

            def pv(mc):
                nc.tensor.matmul(
                    op0, vt[mc][:, h0, :], ess[mc][:, 0:512],
                    start=(mc == 0), stop=(mc == MK - 1),
                )
                nc.tensor.matmul(
                    op1, vt[mc][:, h1, :], ess[mc][:, 512:1024],
                    start=(mc == 0), stop=(mc == MK - 1),
                )

            def proj_slice(slot):
                if not build_next:
                    return
                # each branch allocates ONE psum tile; the extra untouched
                # dummy alloc keeps the 2-buffer pp rotation parity so
                # S-pair(mc) reuses sp(mc-2) (pipeline depth 2), not the
                # tile freed by the previous chunk's exp.
                if slot == 1:
                    ppool.tile([128, 1024], F32, tag="pp", name=f"dq{nxt}")
                    qt[nxt] = qproj_mm(nxt)
                elif slot == 3:
                    kt[nxt] = kpool.tile(
                        [128, M], BF16, tag="kt", name=f"kt{nxt}"
                    )
                    ppool.tile([128, 1024], F32, tag="pp", name=f"da{nxt}")
                    kproj_mm(nxt, kt[nxt], 0)
                elif slot == 5:
                    ppool.tile([128, 1024], F32, tag="pp", name=f"db{nxt}")
                    kproj_mm(nxt, kt[nxt], 1)

            def s_pair(mc):
                sp = ppool.tile([128, 1024], F32, tag="pp")
                nc.tensor.matmul(
                    sp[:, 0:512],
                    ktile[0:64, mc * 128:(mc + 1) * 128],
                    qtile[0:64, :],
                    start=True,
                    stop=True,
                )
                nc.tensor.matmul(
                    sp[:, 512:1024],
                    ktile[64:128, mc * 128:(mc + 1) * 128],
                    qtile[64:128, :],
                    start=True,
                    stop=True,
                )
                es = epool.tile([128, 1024], BF16, tag="es")
                nc.scalar.activation(
                    es, sp, mybir.ActivationFunctionType.Exp, scale=SCALE
                )
                ess[mc] = es

            # S-pair for chunk mc+1 is emitted BEFORE chunk mc's PV and
            # projection work, so those fill the PE's exp-wait shadow
            # instead of delaying the next score matmul (and the exp fed
            # from it).
            s_pair(0)
            for mc in range(MK):
                if mc + 1 < MK:
                    s_pair(mc + 1)
                if build_v:
                    vt[mc] = vproj(mc)
                if mc >= 1:
                    pv(mc - 1)
                proj_slice(mc)
            pv(MK - 1)
            return op0, op1

        def normalize(hp, op0, op1):
            t = opool.tile([128, NSH], BF16, tag="ot", name=f"ot{hp}")
            for j, op in enumerate((op0, op1)):
                zr = zpool.tile([64, 512], F32, tag="z")
                nc.vector.reciprocal(zr, op[64:128, :])
                nc.vector.tensor_tensor(
                    t[j * 64:(j + 1) * 64, :], op[0:64, :], zr,
                    mybir.AluOpType.mult,
                )
            ot[hp] = t

        qt[0] = qproj_mm(0)
        kt[0] = kpool.tile([128, M], BF16, tag="kt", name="kt0")
        kproj_mm(0, kt[0], 0)
        kproj_mm(0, kt[0], 1)
        prev = None
        for hp in range(HP):
            if prev is not None:
                normalize(hp - 1, *prev)
            prev = attn_block(
                hp, build_v=(hp == 0), build_next=(hp < HP - 1)
            )
        normalize(HP - 1, *prev)

        # ---- output projection: two 2-bank PSUM waves.  bp enters as a
        # rank-1 ones x bp-row matmul and the result is DMA'd straight from
        # PSUM, so nothing in the tail needs the DVE after the reciprocals.
        for wave in range(2):
            pss = []
            for n4 in (2 * wave, 2 * wave + 1):
                ps = ppool.tile([128, 1024], F32, tag="pp")
                pss.append(ps)
                for nh in range(2):
                    sl = slice(nh * 512, nh * 512 + 384)
                    for ci in range(CK - 1):
                        nc.tensor.matmul(
                            ps[:, sl],
                            ot[ci][:, n4 * 128:(n4 + 1) * 128],
                            wp[:, ci, nh * 384:(nh + 1) * 384],
                            start=(ci == 0),
                            stop=False,
                        )
                    nc.tensor.matmul(
                        ps[:, sl],
                        ones[:, 0:128],
                        bp_s[:, nh * 384:(nh + 1) * 384],
                        start=False,
                        stop=False,
                    )
            if wave == 0:
                wop = oppool.tile([128, 512], F32, tag="op", name="tailwarm")
                for i in range(16):
                    nc.tensor.matmul(
                        wop, wrm[:, 0:128], wrm,
                        start=(i == 0), stop=(i == 15),
                    )
            for i, n4 in enumerate((2 * wave, 2 * wave + 1)):
                ps = pss[i]
                for nh in range(2):
                    sl = slice(nh * 512, nh * 512 + 384)
                    nc.tensor.matmul(
                        ps[:, sl],
                        ot[CK - 1][:, n4 * 128:(n4 + 1) * 128],
                        wp[:, CK - 1, nh * 384:(nh + 1) * 384],
                        start=False,
                        stop=True,
                    )
                outs = outpool.tile([128, C], F32, tag="outs")
                # ACT is idle in the tail; the copy keeps the DVE free
                # right after the last pair's reciprocals.
                src3 = ps.rearrange("p (h e) -> p h e", e=512)
                nc.scalar.copy(
                    outs.rearrange("p (h e) -> p h e", e=384),
                    src3[:, :, 0:384],
                )
                nc.sync.dma_start(out=out_c[n4], in_=outs)

    if not nc.is_finalized():
        nc.finalize()
    return nc



_NC_CACHE = None


def _get_nc():
    global _NC_CACHE
    if _NC_CACHE is None:
        _NC_CACHE = build_bass()
    return _NC_CACHE


def make_in_maps(x, y, yw, Wq, Wk, Wv, Wp, bp):
    import ml_dtypes

    bf = ml_dtypes.bfloat16
    x = np.asarray(x, np.float32)
    y = np.asarray(y, np.float32)
    yw = np.asarray(yw, np.float32)
    wqT = np.ascontiguousarray(np.asarray(Wq, np.float32).T).astype(bf)
    wkT = np.ascontiguousarray(np.asarray(Wk, np.float32).T).astype(bf)
    wvT = np.ascontiguousarray(np.asarray(Wv, np.float32).T).astype(bf)
    wpT = np.ascontiguousarray(np.asarray(Wp, np.float32).T).astype(bf)
    bpr = np.asarray(bp, np.float32).reshape(1, C).astype(bf)

    in_maps = []
    for c in range(N_CORES):
        b, half = divmod(c, 2)
        n0 = half * NSH
        in_maps.append(
            {
                "xT": np.ascontiguousarray(x[b, n0:n0 + NSH, :].T).astype(bf),
                "yT": np.ascontiguousarray(y[b].T).astype(bf),
                "ywr": np.ascontiguousarray(yw[b].reshape(1, M)).astype(bf),
                "wqT": wqT,
                "wkT": wkT,
                "wvT": wvT,
                "wpT": wpT,
                "bpr": bpr,
            }
        )
    return in_maps


def run(inputs, trace=False):
    """Returns (full_output, BassKernelResults)."""
    from concourse.bass_utils import run_bass_kernel_spmd

    nc = _get_nc()
    in_maps = make_in_maps(**inputs)
    res = run_bass_kernel_spmd(
        nc, in_maps, list(range(N_CORES)), trace=trace
    )
    full = np.empty((B, N, C), dtype=np.float32)
    for c in range(N_CORES):
        b, half = divmod(c, 2)
        n0 = half * NSH
        full[b, n0:n0 + NSH, :] = res.results[c]["out"]
    return full, res


def kernel(**inputs):
    full, _ = run(inputs, trace=False)
    return full


# revision 27
# speedup vs baseline: 1.0663x; 1.0663x over previous
"""Trainium2 Bass kernel for nn_CrossAttention (B=4, N=M=1024, C=768, H=12, D=64).

Sharding: pure data-parallel over 8 cores. Core c handles batch b = c // 2 and
query rows [512*(c%2), 512*(c%2)+512). Each core computes K/V for its batch
(duplicated across the 2 cores sharing a batch) so no collectives are needed.

All-bf16 datapath (fp32 PSUM accumulation); bf16 streams the PE at 1 cycle/row
and avoids the fp32r power throttle. Host-side layout:
  xT  [768, 512]   = x[b, n0:n0+512, :].T   (c-major for Q projection)
  yT  [768, 1024]  = y[b].T                 (c-major for K/V projection)
  w*T [768, 768]   = W.T                    (c-major weights)
  ywr [1, 1024]    = yw[b] row (bf16), bp fp32 row (DMA-replicated to 128
                     partitions for the DVE bias add)

Device dataflow (all matmuls bf16 x bf16 -> fp32 PSUM):
  QT[co,n] = sum_c wqT[c,co] xT[c,n]
  KT[co,m] = sum_c wkT[c,co] yT[c,m] + ones-row x ywr (rank-1 bias matmul)
  V[m,cv]  = sum_c yT[c,m] wvT[c,cv]  in a [128, 12, 128] per-chunk layout
             whose cols 64:128 are memset to 1 so the PV matmul's PSUM rows
             64:128 accumulate Z replicated 64x (softmax denominator, free)
  per head PAIR: two K=64 S-matmuls at PE array tile positions (0,0)/(64,0)
  (they execute concurrently on disjoint sub-arrays) into one [128,1024]
  PSUM tile, ONE exp over [128,1024] -> bf16, two PV matmuls.
  1/Z = stock DVE reciprocal on PSUM rows 64:128 (reciprocal_approx_fast is
  broken on this hardware), DVE multiply -> OT bf16.
  out[n,co] = sum_ci OT[ci,n] wpT[ci,co]; + bp via DVE add (replicated row).

Schedule notes (what the trace iterations taught us):
  - DMA: only use APs whose inner contiguous run is large. Each [128, 768]
    weight chunk of a (k p) n view is one linear 393KB region; the p-major
    single-DMA variant (768B segments) runs at ~2 GB/s/engine and starves
    everything.
  - ~8.5 us of throwaway warm-up matmuls at t=0 keep the HAM activity
    monitor at K=8/8 (2.4 GHz) while the weights load.
  - Block hp runs head-pair hp's 8-chunk attention with head-pair hp+1's
    Q/K projections interleaved at chunks 1/3/5 (V projection inside block
    0), so the PE never idles long enough to re-throttle.
  - Projection PSUM->SBUF casts run on the SCALAR engine: they land in the
    exp FIFO right where the PSUM pool rotation needs them, and keep the
    DVE free for the reciprocals (GpSimd cannot read PSUM; a DVE cast
    behind a 3.4 us reciprocal stalled the PE a full 7 us per block).
  - Output projection runs in two 2-bank PSUM waves, ci=5 last, so only
    the last pair's normalize sits on the critical path.
"""

import sys

for _p in ("/opt/trn_rl_repo",):
    if _p not in sys.path:
        sys.path.insert(0, _p)

import numpy as np
from contextlib import ExitStack

import concourse.bass as bass
import concourse.mybir as mybir
import concourse.tile as tile
from concourse import bacc

F32 = mybir.dt.float32
BF16 = mybir.dt.bfloat16

B = 4
N = 1024
M = 1024
C = 768
H = 12
D = 64
NSH = 512            # query rows per core
CK = C // 128        # 6 chunks of the feature dim
MK = M // 128        # 8 chunks of the key dim
HP = H // 2          # 6 head pairs (one KT/QT co-chunk each)
SCALE = D ** -0.5
N_CORES = 8
N_WARM = 36          # warm-up matmuls to keep HAM at 8/8 during loads


def build_bass():
    nc = bacc.Bacc("TRN2", target_bir_lowering=False, debug=False)

    xT = nc.dram_tensor("xT", [C, NSH], BF16, kind="ExternalInput").ap()
    yT = nc.dram_tensor("yT", [C, M], BF16, kind="ExternalInput").ap()
    ywr = nc.dram_tensor("ywr", [1, M], BF16, kind="ExternalInput").ap()
    wqT = nc.dram_tensor("wqT", [C, C], BF16, kind="ExternalInput").ap()
    wkT = nc.dram_tensor("wkT", [C, C], BF16, kind="ExternalInput").ap()
    wvT = nc.dram_tensor("wvT", [C, C], BF16, kind="ExternalInput").ap()
    wpT = nc.dram_tensor("wpT", [C, C], BF16, kind="ExternalInput").ap()
    bpr = nc.dram_tensor("bpr", [1, C], BF16, kind="ExternalInput").ap()
    out = nc.dram_tensor("out", [NSH, C], F32, kind="ExternalOutput").ap()

    # k-major chunk views: each [128, x] chunk is one contiguous DRAM region
    wq_c = wqT.rearrange("(k p) n -> k p n", p=128)
    wk_c = wkT.rearrange("(k p) n -> k p n", p=128)
    wv_c = wvT.rearrange("(k p) n -> k p n", p=128)
    wp_c = wpT.rearrange("(k p) n -> k p n", p=128)
    xT_c = xT.rearrange("(k p) n -> k p n", p=128)
    yT_c = yT.rearrange("(k p) n -> k p n", p=128)
    out_c = out.rearrange("(k p) n -> k p n", p=128)

    with tile.TileContext(nc) as tc, ExitStack() as ctx:
        wpool = ctx.enter_context(tc.tile_pool(name="w", bufs=4))
        cpool = ctx.enter_context(tc.tile_pool(name="const", bufs=1))
        qpool = ctx.enter_context(tc.tile_pool(name="qt", bufs=3))
        kpool = ctx.enter_context(tc.tile_pool(name="kt", bufs=3))
        vpool = ctx.enter_context(tc.tile_pool(name="vs", bufs=MK))
        opool = ctx.enter_context(tc.tile_pool(name="ot", bufs=CK))
        epool = ctx.enter_context(tc.tile_pool(name="es", bufs=3))
        outpool = ctx.enter_context(tc.tile_pool(name="outs", bufs=2))
        zpool = ctx.enter_context(tc.tile_pool(name="z", bufs=4))
        ppool = ctx.enter_context(tc.tile_pool(name="pp", bufs=2, space="PSUM"))
        oppool = ctx.enter_context(tc.tile_pool(name="op", bufs=3, space="PSUM"))
        pjpool = ctx.enter_context(tc.tile_pool(name="pj", bufs=1, space="PSUM"))

        # ---- PE warm-up: throwaway matmuls with no DMA dependency ----
        wrm = cpool.tile([128, 512], BF16, tag="wrm")
        nc.gpsimd.memset(wrm, 0.0)
        ones = cpool.tile([1, C], BF16, tag="ones")
        nc.gpsimd.memset(ones, 1.0)
        wps = ppool.tile([128, 1024], F32, tag="pp", name="warmps")
        for i in range(N_WARM):
            nc.tensor.matmul(
                wps[:, 0:512], wrm[:, 0:128], wrm,
                start=(i == 0), stop=(i == N_WARM - 1),
            )

        # ---- input loads: contiguous per-chunk DMAs, 2 HWDGE rings ----
        # scalar ring: wq x6 (Q proj gate), yT x6, wp x6
        # sync ring:   xt x6, yw row, wk x6, wv x6, bp replicate
        wq = wpool.tile([128, CK, C], BF16, tag="w", name="wq")
        xt = cpool.tile([128, CK, NSH], BF16, tag="xt")
        for i in range(CK):
            nc.scalar.dma_start(out=wq[:, i, :], in_=wq_c[i])
            nc.sync.dma_start(out=xt[:, i, :], in_=xT_c[i])
        yw_s = cpool.tile([1, M], BF16, tag="yws")
        nc.sync.dma_start(out=yw_s, in_=ywr)
        yt = cpool.tile([128, CK, M], BF16, tag="yt")
        wk = wpool.tile([128, CK, C], BF16, tag="w", name="wk")
        for i in range(CK):
            nc.scalar.dma_start(out=yt[:, i, :], in_=yT_c[i])
            nc.sync.dma_start(out=wk[:, i, :], in_=wk_c[i])
        wv = wpool.tile([128, CK, C], BF16, tag="w", name="wv")
        for i in range(CK):
            nc.sync.dma_start(out=wv[:, i, :], in_=wv_c[i])
        # wp is not needed until the output projection: load it via the
        # gpsimd SWDGE ring so it costs neither HWDGE ring any bandwidth
        # and adds no issue time on the ACT queue.
        wp = wpool.tile([128, CK, C], BF16, tag="w", name="wp")
        for i in range(CK):
            nc.gpsimd.dma_start(out=wp[:, i, :], in_=wp_c[i])
        bp_s = cpool.tile([1, C], BF16, tag="bps")
        nc.sync.dma_start(out=bp_s, in_=bpr)
        # preload the ACT exp table from the memset ones tile: zero DMA
        # dependency, so the table load cannot block the projection casts
        # queued behind it on the ACT engine.
        warm = cpool.tile([1, 8], BF16, tag="warm")
        nc.scalar.activation(
            warm, ones[0:1, 0:8], mybir.ActivationFunctionType.Exp,
            scale=SCALE,
        )

        def qproj_mm(co):
            ps = pjpool.tile([128, 512], F32, tag="pj")
            for ci in range(CK):
                nc.tensor.matmul(
                    ps,
                    wq[:, ci, co * 128:(co + 1) * 128],
                    xt[:, ci, :],
                    start=(ci == 0),
                    stop=(ci == CK - 1),
                )
            t = qpool.tile([128, NSH], BF16, tag="qt")
            # ACT cast: lands in the exp FIFO early enough to release the
            # 1-bank proj psum for the next projection group (a DVE cast
            # would sit behind the 8.5us of reciprocals).
            nc.scalar.copy(t, ps)
            return t

        def kproj_mm(co, t, mh):
            ps = pjpool.tile([128, 512], F32, tag="pj")
            sl = slice(mh * 512, (mh + 1) * 512)
            for ci in range(CK):
                nc.tensor.matmul(
                    ps,
                    wk[:, ci, co * 128:(co + 1) * 128],
                    yt[:, ci, sl],
                    start=(ci == 0),
                    stop=False,
                )
            # += ones-row^T x yw-row: the additive key bias, rank-1
            nc.tensor.matmul(
                ps,
                ones[:, co * 128:(co + 1) * 128],
                yw_s[:, sl],
                start=False,
                stop=True,
            )
            nc.scalar.copy(t[:, sl], ps)

        def vproj(mc):
            t = vpool.tile([128, H, 128], BF16, tag="vs")
            nc.gpsimd.memset(t[:, :, 64:128], 1.0)
            ps = ppool.tile([128, 1024], F32, tag="pp")
            for nh in range(2):
                sl = slice(nh * 512, nh * 512 + 384)
                for ci in range(CK):
                    nc.tensor.matmul(
                        ps[:, sl],
                        yt[:, ci, mc * 128:(mc + 1) * 128],
                        wv[:, ci, nh * 384:(nh + 1) * 384],
                        start=(ci == 0),
                        stop=(ci == CK - 1),
                    )
            for nh in range(2):
                src = ps[:, nh * 512:nh * 512 + 384].rearrange(
                    "p (h e) -> p h e", e=64
                )
                nc.vector.tensor_copy(t[:, nh * 6:(nh + 1) * 6, 0:64], src)
            return t

        vt = [None] * MK
        ot = [None] * HP
        qt = [None] * HP
        kt = [None] * HP

        def attn_block(hp, build_v, build_next):
            """Head-pair hp's attention; next pair's projections (and, for
            hp==0, the V projection) sliced into the chunk loop.  The PV
            pair for chunk mc is emitted during chunk mc+1 so the PE never
            sits behind the in-flight exp."""
            h0, h1 = 2 * hp, 2 * hp + 1
            qtile, ktile = qt[hp], kt[hp]
            op0 = oppool.tile([128, 512], F32, tag="op", name=f"op{h0}")
            op1 = oppool.tile([128, 512], F32, tag="op", name=f"op{h1}")
            nxt = hp + 1
            ess = [None] * MK

            def pv(mc):
                nc.tensor.matmul(
                    op0, vt[mc][:, h0, :], ess[mc][:, 0:512],
                    start=(mc == 0), stop=(mc == MK - 1),
                )
                nc.tensor.matmul(
                    op1, vt[mc][:, h1, :], ess[mc][:, 512:1024],
                    start=(mc == 0), stop=(mc == MK - 1),
                )

            def proj_slice(slot):
                if not build_next:
                    return
                # each branch allocates ONE psum tile; the extra untouched
                # dummy alloc keeps the 2-buffer pp rotation parity so
                # S-pair(mc) reuses sp(mc-2) (pipeline depth 2), not the
                # tile freed by the previous chunk's exp.
                if slot == 1:
                    qt[nxt] = qproj_mm(nxt)
                elif slot == 3:
                    kt[nxt] = kpool.tile(
                        [128, M], BF16, tag="kt", name=f"kt{nxt}"
                    )
                    kproj_mm(nxt, kt[nxt], 0)
                elif slot == 5:
                    kproj_mm(nxt, kt[nxt], 1)

            def s_pair(mc):
                sp = ppool.tile([128, 1024], F32, tag="pp")
                nc.tensor.matmul(
                    sp[:, 0:512],
                    ktile[0:64, mc * 128:(mc + 1) * 128],
                    qtile[0:64, :],
                    start=True,
                    stop=True,
                )
                nc.tensor.matmul(
                    sp[:, 512:1024],
                    ktile[64:128, mc * 128:(mc + 1) * 128],
                    qtile[64:128, :],
                    start=True,
                    stop=True,
                )
                es = epool.tile([128, 1024], BF16, tag="es")
                nc.scalar.activation(
                    es, sp, mybir.ActivationFunctionType.Exp, scale=SCALE
                )
                ess[mc] = es

            # S-pair for chunk mc+1 is emitted BEFORE chunk mc's PV and
            # projection work, so those fill the PE's exp-wait shadow
            # instead of delaying the next score matmul (and the exp fed
            # from it).
            s_pair(0)
            for mc in range(MK):
                if mc + 1 < MK:
                    s_pair(mc + 1)
                if build_v:
                    vt[mc] = vproj(mc)
                if mc >= 1:
                    pv(mc - 1)
                proj_slice(mc)
            pv(MK - 1)
            return op0, op1

        def normalize(hp, op0, op1):
            t = opool.tile([128, NSH], BF16, tag="ot", name=f"ot{hp}")
            for j, op in enumerate((op0, op1)):
                # quick 0.65us drain to SBUF frees the PSUM bank long
                # before the 3.4us reciprocal runs (op pool: 3 banks).
                zc = zpool.tile([128, 512], F32, tag="zc")
                nc.vector.tensor_copy(zc, op)
                zr = zpool.tile([64, 512], F32, tag="z")
                nc.vector.reciprocal(zr, zc[64:128, :])
                nc.vector.tensor_tensor(
                    t[j * 64:(j + 1) * 64, :], zc[0:64, :], zr,
                    mybir.AluOpType.mult,
                )
            ot[hp] = t

        qt[0] = qproj_mm(0)
        kt[0] = kpool.tile([128, M], BF16, tag="kt", name="kt0")
        kproj_mm(0, kt[0], 0)
        kproj_mm(0, kt[0], 1)
        prev = None
        for hp in range(HP):
            if prev is not None:
                normalize(hp - 1, *prev)
            prev = attn_block(
                hp, build_v=(hp == 0), build_next=(hp < HP - 1)
            )
        normalize(HP - 1, *prev)

        # ---- output projection: two 2-bank PSUM waves.  bp enters as a
        # rank-1 ones x bp-row matmul and the result is DMA'd straight from
        # PSUM, so nothing in the tail needs the DVE after the reciprocals.
        for wave in range(2):
            pss = []
            for n4 in (2 * wave, 2 * wave + 1):
                ps = ppool.tile([128, 1024], F32, tag="pp")
                pss.append(ps)
                for nh in range(2):
                    sl = slice(nh * 512, nh * 512 + 384)
                    for ci in range(CK - 1):
                        nc.tensor.matmul(
                            ps[:, sl],
                            ot[ci][:, n4 * 128:(n4 + 1) * 128],
                            wp[:, ci, nh * 384:(nh + 1) * 384],
                            start=(ci == 0),
                            stop=False,
                        )
                    nc.tensor.matmul(
                        ps[:, sl],
                        ones[:, 0:128],
                        bp_s[:, nh * 384:(nh + 1) * 384],
                        start=False,
                        stop=False,
                    )
            if wave == 0:
                wop = oppool.tile([128, 512], F32, tag="op", name="tailwarm")
                for i in range(16):
                    nc.tensor.matmul(
                        wop, wrm[:, 0:128], wrm,
                        start=(i == 0), stop=(i == 15),
                    )
            for i, n4 in enumerate((2 * wave, 2 * wave + 1)):
                ps = pss[i]
                for nh in range(2):
                    sl = slice(nh * 512, nh * 512 + 384)
                    nc.tensor.matmul(
                        ps[:, sl],
                        ot[CK - 1][:, n4 * 128:(n4 + 1) * 128],
                        wp[:, CK - 1, nh * 384:(nh + 1) * 384],
                        start=False,
                        stop=True,
                    )
                outs = outpool.tile([128, C], F32, tag="outs")
                # ACT is idle in the tail; the copy keeps the DVE free
                # right after the last pair's reciprocals.
                src3 = ps.rearrange("p (h e) -> p h e", e=512)
                nc.scalar.copy(
                    outs.rearrange("p (h e) -> p h e", e=384),
                    src3[:, :, 0:384],
                )
                nc.sync.dma_start(out=out_c[n4], in_=outs)

    if not nc.is_finalized():
        nc.finalize()
    return nc



_NC_CACHE = None


def _get_nc():
    global _NC_CACHE
    if _NC_CACHE is None:
        _NC_CACHE = build_bass()
    return _NC_CACHE


def make_in_maps(x, y, yw, Wq, Wk, Wv, Wp, bp):
    import ml_dtypes

    bf = ml_dtypes.bfloat16
    x = np.asarray(x, np.float32)
    y = np.asarray(y, np.float32)
    yw = np.asarray(yw, np.float32)
    wqT = np.ascontiguousarray(np.asarray(Wq, np.float32).T).astype(bf)
    wkT = np.ascontiguousarray(np.asarray(Wk, np.float32).T).astype(bf)
    wvT = np.ascontiguousarray(np.asarray(Wv, np.float32).T).astype(bf)
    wpT = np.ascontiguousarray(np.asarray(Wp, np.float32).T).astype(bf)
    bpr = np.asarray(bp, np.float32).reshape(1, C).astype(bf)

    in_maps = []
    for c in range(N_CORES):
        b, half = divmod(c, 2)
        n0 = half * NSH
        in_maps.append(
            {
                "xT": np.ascontiguousarray(x[b, n0:n0 + NSH, :].T).astype(bf),
                "yT": np.ascontiguousarray(y[b].T).astype(bf),
                "ywr": np.ascontiguousarray(yw[b].reshape(1, M)).astype(bf),
                "wqT": wqT,
                "wkT": wkT,
                "wvT": wvT,
                "wpT": wpT,
                "bpr": bpr,
            }
        )
    return in_maps


def run(inputs, trace=False):
    """Returns (full_output, BassKernelResults)."""
    from concourse.bass_utils import run_bass_kernel_spmd

    nc = _get_nc()
    in_maps = make_in_maps(**inputs)
    res = run_bass_kernel_spmd(
        nc, in_maps, list(range(N_CORES)), trace=trace
    )
    full = np.empty((B, N, C), dtype=np.float32)
    for c in range(N_CORES):
        b, half = divmod(c, 2)
        n0 = half * NSH
        full[b, n0:n0 + NSH, :] = res.results[c]["out"]
    return full, res


def kernel(**inputs):
    full, _ = run(inputs, trace=False)
    return full
